# revision 39
# baseline (speedup 1.0000x reference)
"""Trainium2 Bass kernel for nn_BaseVectorQuantizer (vq_codebook).

Reference computation (per token x of dim 32, codebook C [256, 32]):
    idx     = argmin_k ||x - C_k||^2
    counts  = bincount(idx, 256)
    rotated = ||x|| * l2norm(C)[idx]        (exact rotation-trick identity)

Device strategy (8 NeuronCores, data-parallel over the 524288 tokens):
  - Host pre-transposes x and splits it (and 2*C^T) into bf16 hi/lo pairs;
    a single K=98 bf16 matmul per 128-token tile produces fp32-accurate
    scores  S[tok, k] = 2*x.c_k - |c_k|^2  in PSUM (3-term bf16 product
    expansion + split bias rows).
  - VectorE: per 4-tile group one reduce_max, then per tile one fused
    scalar_tensor_tensor pass: OHK = (S >= max) * (k+1) (bf16, to SBUF)
    with accum_out = idx+1.
  - PE: histogram matmul ones^T @ OHK accumulated over all tiles into one
    PSUM bank (counts * (k+1); host divides), and a one-hot gather
    OHK^T @ (l2norm(C)/(k+1)) via PE transpose + ScalarE copy, yielding
    l2norm(C)[idx] in PSUM; ScalarE scales by ||x|| while evacuating.
  - Host: reassembles shards, divides the histogram by (k+1), sums the 8
    per-core histograms (the all-reduce), and inverse-permutes tokens.
"""

import os

import numpy as np
import ml_dtypes

import concourse.bass as bass
import concourse.mybir as mybir
import concourse.tile as tile
from concourse.bass_utils import run_bass_kernel_spmd
from concourse.masks import make_identity
from concourse.tile_rust import add_dep_helper
from concourse.tile_sem_assignment import N_PROCS, PROC_NAME_TO_IDX

# The kernel-tail drain waits on every sem lane ever used, but the SP
# CTRL_NO struct fits only ONE sync-wait command, so walrus rejects it.
# Emit one single-wait drain per sem lane instead.  Input (DMAHW) lane
# completions are implied by the engine waits (every loaded byte was
# consumed by an engine that is itself drained), so skip those lanes.
from concourse.vector_clock import ScopedClock as _ScopedClock

_DMAHW_PROCS = {i for n, i in PROC_NAME_TO_IDX.items() if n.startswith("DMAHW")}


def _patched_dab(self, tick_clock, wait_clock):
    gc = tick_clock.global_clock
    VC = type(gc)
    for p in range(N_PROCS):
        if p in _DMAHW_PROCS:
            continue
        v = gc[p]
        if not v:
            continue
        vals = [0] * N_PROCS
        vals[p] = v
        d = self.nc.sync.drain()
        wait_clock.add_sem_waits(d.ins, _ScopedClock({None: VC(vals)}))
    self.nc.all_engine_barrier()
    assert self.sems is not None
    popped = self.nc._tile_sem_poison_stack.pop()
    assert popped is self._sem_poison
    self.nc.clear_and_free_semaphores(list(self.sems.allocated().values()))
    self.nc.all_engine_barrier()


tile.TileContext._drain_and_barrier = _patched_dab

BF16 = ml_dtypes.bfloat16
K = 256          # codebook size
D = 32           # embed dim
N_CORES = 8
N_TOKENS = 524288
T_CORE = N_TOKENS // N_CORES   # 65536 tokens per core
GS = 4           # tiles per reduce group
RCH = 128        # tiles per rotated output chunk
XCH = 64         # tiles per xstack DMA chunk
OHK_BUFS = 12    # ohk slot depth (WAR distance for the DVE fence)

_f32 = mybir.dt.float32
_bf16 = mybir.dt.bfloat16
_i32 = mybir.dt.int32


def build_module(T: int, gs: int = GS, rch: int = RCH, xch: int = XCH):
    """Build the per-core Bass module for T tokens (T % (128*gs) == 0)."""
    NT = T // 128                      # token tiles
    assert NT % gs == 0
    rch = min(rch, NT)
    xch = min(xch, NT)
    assert NT % rch == 0 and NT % xch == 0

    nc = bass.Bass("TRN2", target_bir_lowering=False, debug=False)

    # ---- DRAM I/O ----
    xstack = nc.dram_tensor("xstack", [98, T], _bf16, kind="ExternalInput")
    wmat_d = nc.dram_tensor("wmat", [98, K], _bf16, kind="ExternalInput")
    kval_d = nc.dram_tensor("kval", [128, K], _f32, kind="ExternalInput")
    cng_d = nc.dram_tensor("cng", [128, 4 * D], _bf16, kind="ExternalInput")
    nxt_d = nc.dram_tensor("nxt", [128, NT], _f32, kind="ExternalInput")
    n_rch = max(NT // min(rch, NT), 1)
    rot_ds = [nc.dram_tensor(f"rot{c}", [128, min(rch, NT) * D], _f32,
                             kind="ExternalOutput") for c in range(n_rch)]
    idx_d = nc.dram_tensor("idx", [128, NT], _i32, kind="ExternalOutput")
    hist_d = nc.dram_tensor("hist", [1, K], _f32, kind="ExternalOutput")

    with tile.TileContext(nc) as tc, tc.tile_pool(name="const", bufs=1) as cpool, \
         tc.tile_pool(name="xstk", bufs=1) as xpool, \
         tc.tile_pool(name="spsum", bufs=2, space="PSUM") as spool, \
         tc.tile_pool(name="tpsum", bufs=2, space="PSUM") as tpool, \
         tc.tile_pool(name="gpsum", bufs=1, space="PSUM") as gpool, \
         tc.tile_pool(name="hpsum", bufs=1, space="PSUM") as hpool, \
         tc.tile_pool(name="mred", bufs=3) as mpool, \
         tc.tile_pool(name="ohk", bufs=1) as opool, \
         tc.tile_pool(name="ohkT", bufs=2) as otpool, \
         tc.tile_pool(name="plane", bufs=1) as ppool:

        # ---- constants (packed into one bf16 + one f32 tile to dodge
        # the 4KB-per-tile SBUF padding) ----
        cb = cpool.tile([128, 704], _bf16, tag="cb")
        wmat = cb[:98, 0:K]
        ident = cb[:, K:K + 128]
        cng = cb[:, K + 128:K + 128 + 4 * D]
        ones_t = cb[:, K + 128 + 4 * D:K + 128 + 4 * D + 1]
        nc.sync.dma_start(wmat, wmat_d[:, :])
        nc.sync.dma_start(cng, cng_d[:, :])
        nc.gpsimd.memset(ones_t, 1.0)
        make_identity(nc, ident)
        cf = cpool.tile([128, K + NT + 8 + K], _f32, tag="cf")
        kval = cf[:, 0:K]
        nxt = cf[:, K:K + NT]
        touch = cf[0:1, K + NT:K + NT + 4]
        nc.sync.dma_start(kval, kval_d[:, :])
        nc.sync.dma_start(nxt, nxt_d[:, :])

        # Pre-touch const tiles on their consuming engines: the STT /
        # ACTIVATE ISA structs have very few sync-wait slots, so the const
        # DMA-lane waits must be absorbed by these early touches instead of
        # riding on the hot-loop instructions.
        nc.vector.tensor_copy(touch[:, 0:1], kval[0:1, 0:1])
        nc.scalar.copy(touch[:, 1:2], nxt[0:1, 0:1])
        nc.vector.tensor_copy(touch[:, 2:3], wmat[0:1, 0:1])
        nc.scalar.copy(touch[:, 3:4], cng[0:1, 0:1])


        # ---- planes ----
        idxp = ppool.tile([128, NT], _f32)
        mplane = ppool.tile([128, 16 * gs], _f32)   # mgrp ring (16 groups)
        otbuf = ppool.tile([128, 2 * 256], _bf16)   # ohkT ring (2 slots)
        rbuf = ppool.tile([128, 2 * rch * D], _f32)  # rotated ring (2 slots)
        hist_ps = hpool.tile([1, K], _f32)
        # explicit circular buffer for the one-hot tiles: deterministic WAR
        # distance so the per-group DVE fence can absorb the PE waits (the
        # STT ISA struct only has one sync-wait slot)
        obuf = opool.tile([128, OHK_BUFS * K], _bf16)

        def lhsT(j):
            return xres[:, j * 128:(j + 1) * 128]

        # resident x stack: disjoint-slice chunk loads (no slot recycling,
        # so the loads carry no WAR/WAW waits)
        xres = xpool.tile([98, NT * 128], _bf16, tag="xres")
        for c in range((NT + xch - 1) // xch):
            nc.sync.dma_start(xres[:, c * xch * 128:(c + 1) * xch * 128],
                              xstack[:, c * xch * 128:(c + 1) * xch * 128])

        rchunk = None
        ohk_readers = {}    # tile j -> PE instrs reading its ohk slot
        rot_dmas = {}       # chunk c -> rot DMA instruction
        for g in range(NT // gs):
            sgrp = spool.tile([128, gs * K], _f32)
            for t in range(gs):
                j = g * gs + t
                nc.tensor.matmul(sgrp[:, t * K:(t + 1) * K], lhsT(j),
                                 wmat, start=True, stop=True)
            mgrp = mplane[:, (g % 16) * gs:(g % 16) * gs + gs]
            nc.vector.tensor_reduce(
                mgrp, sgrp[:].rearrange("p (g n) -> p g n", n=K),
                axis=mybir.AxisListType.X, op=mybir.AluOpType.max)
            # DVE fence absorbing the WAR deps of the ohk slots this group's
            # STTs will recycle — the STT ISA struct has only one sync-wait
            # slot, so the PE wait must land here instead.
            old = [r for t in range(gs)
                   for r in ohk_readers.pop(g * gs + t - OHK_BUFS, [])]
            fence = None
            if old:
                fence = nc.vector.drain()
                for r in old:
                    add_dep_helper(fence.ins, r, reason="ohk WAR fence")

            afence = None
            for t in range(gs):
                j = g * gs + t
                if j % rch == 0:
                    if (c := j // rch) >= 2 and (c - 2) in rot_dmas:
                        o = 600 + 2 * c
                        afence = nc.scalar.copy(cb[0:1, o:o + 1],
                                                cb[0:1, 700:701])
                        add_dep_helper(afence.ins, rot_dmas.pop(c - 2),
                                       reason="rchunk WAR fence")
                    rchunk = rbuf[:, (j // rch % 2) * rch * D:
                                  (j // rch % 2 + 1) * rch * D]
                s = (j % OHK_BUFS) * K
                stt = nc.vector.scalar_tensor_tensor(
                    out=obuf[:, s:s + K], in0=sgrp[:, t * K:(t + 1) * K],
                    scalar=mgrp[:, t:t + 1], in1=kval,
                    op0=mybir.AluOpType.is_ge, op1=mybir.AluOpType.mult,
                    accum_out=idxp[:, j:j + 1])
                # histogram: hist += ones^T @ OHK   (values (k+1)*count_k)
                hmm = nc.tensor.matmul(hist_ps[:], ones_t, obuf[:, s:s + K],
                                       start=(j == 0), stop=(j == NT - 1),
                                       skip_group_check=True)
                # one-hot transpose (PE) -> PSUM bf16, evacuate via ScalarE
                tgrp = tpool.tile([128, 2 * 128], _bf16)
                tr0 = nc.tensor.transpose(tgrp[:, 0:128], obuf[:, s:s + 128],
                                          ident)
                tr1 = nc.tensor.transpose(tgrp[:, 128:256],
                                          obuf[:, s + 128:s + 256], ident)
                ohk_readers[j] = [hmm.ins, tr0.ins, tr1.ins]
                ot = (j % 2) * 256
                ohkT = otbuf[:, ot:ot + 256]
                nc.scalar.copy(ohkT[:, 0:128], tgrp[:, 0:128])
                nc.scalar.copy(ohkT[:, 128:256], tgrp[:, 128:256])
                # gather: q = sum_k OHK[k]*Cn_k/(k+1)  (4 accumulating mms)
                gout = gpool.tile([128, D], _f32)
                nc.tensor.matmul(gout[:], ohkT[:, 0:128], cng[:, 0:D],
                                 start=True, stop=False, skip_group_check=True)
                nc.tensor.matmul(gout[:], ohkT[:, 0:128], cng[:, D:2 * D],
                                 start=False, stop=False, skip_group_check=True)
                nc.tensor.matmul(gout[:], ohkT[:, 128:256], cng[:, 2 * D:3 * D],
                                 start=False, stop=False, skip_group_check=True)
                nc.tensor.matmul(gout[:], ohkT[:, 128:256], cng[:, 3 * D:4 * D],
                                 start=False, stop=True, skip_group_check=True)
                # rotated = nx * q  (ScalarE scale during PSUM->SBUF copy)
                r = j % rch
                smul = nc.scalar.mul(rchunk[:, r * D:(r + 1) * D], gout[:],
                                     nxt[:, j:j + 1])
                if afence is not None:
                    afence = None
                if (j + 1) % rch == 0:
                    rdma = nc.gpsimd.dma_start(rot_ds[j // rch][:, :], rchunk)
                    rot_dmas[j // rch] = rdma.ins

        # ---- finalize indices (accum holds idx+1; SWDGE casts to i32) ----
        nc.vector.tensor_scalar_add(idxp[:], idxp[:], -1.0)
        nc.gpsimd.dma_start(idx_d[:, :], idxp[:])

        # ---- histogram out (PSUM -> SBUF -> DRAM) ----
        hist_sb = cf[0:1, K + NT + 8:K + NT + 8 + K]
        nc.vector.tensor_copy(hist_sb, hist_ps[:])
        nc.gpsimd.dma_start(hist_d[:, :], hist_sb)

    return nc


def host_prep(x: np.ndarray, codebook: np.ndarray):
    """Build per-core input maps. x [N, 32] fp32, codebook [256, 32] fp32."""
    n = x.shape[0]
    t_core = n // N_CORES

    # score weights: rows pair with xstack rows [xh, xl, xh, 1, 1]
    W = (2.0 * codebook.T).astype(np.float32)            # [32, 256]
    Ch = W.astype(BF16)
    Cl = (W - Ch.astype(np.float32)).astype(BF16)
    b = (-(codebook.astype(np.float32) ** 2).sum(-1)).astype(np.float32)
    bh = b.astype(BF16)
    bl = (b - bh.astype(np.float32)).astype(BF16)
    wmat = np.concatenate([Ch, Ch, Cl, bh[None, :], bl[None, :]], axis=0)
    wmat = np.ascontiguousarray(wmat, dtype=BF16)        # [98, 256]

    kval = np.broadcast_to(np.arange(1, K + 1, dtype=np.float32), (128, K))
    kval = np.ascontiguousarray(kval)

    cn = codebook / np.maximum(
        np.sqrt((codebook * codebook).sum(-1, keepdims=True)), 1e-6)
    cn = cn.astype(np.float32)
    cnp = cn / np.arange(1, K + 1, dtype=np.float32)[:, None]   # Cn/(k+1)
    gh = cnp.astype(BF16)
    gl = (cnp - gh.astype(np.float32)).astype(BF16)
    cng = np.concatenate(
        [gh[0:128], gl[0:128], gh[128:256], gl[128:256]], axis=1)
    cng = np.ascontiguousarray(cng, dtype=BF16)          # [128, 128]

    nx = np.sqrt((x.astype(np.float32) ** 2).sum(-1)).astype(np.float32)  # [N]

    in_maps = []
    for c in range(N_CORES):
        xs = x[c * t_core:(c + 1) * t_core].astype(np.float32)   # [T, 32]
        xh = xs.astype(BF16)
        xl = (xs - xh.astype(np.float32)).astype(BF16)
        xhT = np.ascontiguousarray(xh.T)                  # [32, T]
        xlT = np.ascontiguousarray(xl.T)
        onesr = np.ones((2, t_core), dtype=BF16)
        xstack = np.concatenate([xhT, xlT, xhT, onesr], axis=0)  # [98, T]
        nxc = nx[c * t_core:(c + 1) * t_core]
        nxt = np.ascontiguousarray(nxc.reshape(-1, 128).T)       # [128, NT]
        in_maps.append({
            "xstack": np.ascontiguousarray(xstack, dtype=BF16),
            "wmat": wmat,
            "kval": kval,
            "cng": cng,
            "nxt": nxt,
        })
    return in_maps


def assemble(results, t_core: int):
    """Reassemble full outputs from per-core result dicts."""
    nt = t_core // 128
    rch = min(RCH, nt)
    rots, idxs = [], []
    hist = np.zeros(K, dtype=np.float64)
    for res in results:
        rot = np.concatenate(
            [np.asarray(res[f"rot{c}"]) for c in range(max(nt // rch, 1))],
            axis=1).reshape(128, nt, D)
        rots.append(np.ascontiguousarray(rot.transpose(1, 0, 2)).reshape(-1, D))
        idx = np.asarray(res["idx"]).reshape(128, nt)
        idxs.append(np.ascontiguousarray(idx.T).reshape(-1))
        hist += np.asarray(res["hist"]).reshape(K).astype(np.float64)
    rotated = np.concatenate(rots, axis=0).astype(np.float32)
    indices = np.concatenate(idxs, axis=0).astype(np.int32)
    counts = (hist / np.arange(1, K + 1, dtype=np.float64))
    counts = counts.round().astype(np.float32)
    return rotated, indices, counts


_TRACE_RESULTS = {}


def kernel(x: np.ndarray, codebook: np.ndarray):
    x = np.asarray(x, dtype=np.float32)
    codebook = np.asarray(codebook, dtype=np.float32)
    t_core = x.shape[0] // N_CORES
    nc = build_module(t_core)
    in_maps = host_prep(x, codebook)
    trace = bool(int(os.environ.get("BASS_VQ_TRACE", "0")))
    out = run_bass_kernel_spmd(nc, in_maps, core_ids=list(range(N_CORES)),
                               trace=trace)
    _TRACE_RESULTS["last"] = out
    return assemble(out.results, t_core)
